# revision 14
# baseline (speedup 1.0000x reference)
"""DC_CE_Marginal_loss for Trainium2 — 8-core data-parallel Bass kernel, v2.

Shards the [B,C,D,H,W] volume along D across 8 NeuronCores, two launches:

  Launch A (counts): per-(b,c) voxel counts from the one-hot target.
      t is shipped as fp8 (0/1 exact); the whole reduction runs on the
      otherwise-idle TensorEngine as ones-vector matmuls accumulating in
      PSUM (sequential per-plane groups in one bank), then one ACT
      copy+accum -> [16,1] -> DRAM. No DVE streaming work at all.

  Launch B (main loss): compiled specialized on the per-sample present
      counts (n0, n1) derived from launch A. The host permutes channels
      present-first, so absent channels are skipped everywhere and no
      mask tensors are needed. All big elementwise ops are bf16 with
      flat innermost-contiguous APs so the DVE runs in 2x_1p mode:
        bg merge (absent logits -> class 0), wide exp (ACT), pairwise
        S-tree, fast reciprocal, q = e*r via a stride-0 broadcast TT,
        tq = t*q, u-term via fused tensor_tensor_reduce.
      Per-class seg/intersect reductions run on the TensorEngine as
      ones-matmuls into PSUM ([16,FCH] per chunk, sequential groups),
      accumulated into an SBUF [32, MMN] tile; ln(S+pad) is batched at
      the end on ACT (one table set with exp). Host sums the partial
      columns and finishes the loss.
"""
import os
import numpy as np
import ml_dtypes

B, C, D, H, W = 2, 8, 64, 160, 160
NCORES = 8
DS = D // NCORES            # depth slices per core
PLANE = DS * H * W          # voxels per (b,c) plane per core = 204800
P = 128
FREE = PLANE // P           # 1600
NCH = 2                     # chunks per sample plane
FCH = FREE // NCH           # 800
MMN = 400                   # matmul moving free dim (<=512), FCH % MMN == 0
NSUB = FCH // MMN           # sub-matmuls per class per chunk
NVOX = B * D * H * W

NCOL = B * NCH              # lse col per (b, chunk)

U_TTR = os.environ.get("K_U_TTR", "0") == "1"  # TTR dies on HW (runtime INTERNAL)
Q_BCAST = os.environ.get("K_Q_BCAST", "1") == "1"

_CACHE = {}


def _build_a():
    import concourse.bacc as bacc
    import concourse.tile as tile
    from concourse import mybir

    FA = mybir.ActivationFunctionType
    f32 = mybir.dt.float32
    f8 = mybir.dt.float8e4

    nc = bacc.Bacc("TRN2", num_devices=NCORES, name="loss_counts_v2")
    # [P, B*C*FREE]: one 25.6KB contiguous DMA row per partition
    t8 = nc.dram_tensor("t8", [P, B * C, FREE], f8, kind="ExternalInput")
    out = nc.dram_tensor("cnt", [P, B * C], f32, kind="ExternalOutput")

    # f-blocks per plane: lhsT = t[:, blk] (m<=128), rhs = ones -> out
    # [m, 1] accumulated into the plane's own psum column. One bank,
    # sequential per-plane accumulation groups.
    blks = [(j * P, min(P, FREE - j * P)) for j in range((FREE + P - 1) // P)]
    NSEG = 4  # DMA in 4 segments of 4 planes so matmuls start early
    with tile.TileContext(nc) as tc:
        with (
            tc.tile_pool(name="tin", bufs=2) as tin,
            tc.tile_pool(name="cpool", bufs=1) as cpool,
            tc.psum_pool(name="pp", bufs=1) as pp,
        ):
            ones = cpool.tile([P, 1], f8)
            nc.vector.memset(ones[:], 1.0)
            ps = pp.tile([P, B * C], f32)
            PLSEG = B * C // NSEG
            for seg in range(NSEG):
                t_sb = tin.tile([P, PLSEG, FREE], f8, tag="t")
                nc.sync.dma_start(
                    t_sb[:], t8[:, seg * PLSEG : (seg + 1) * PLSEG, :])
                for p in range(PLSEG):
                    bc = seg * PLSEG + p
                    for j, (f0, m) in enumerate(blks):
                        nc.tensor.matmul(
                            ps[0:m, bc : bc + 1],
                            t_sb[:, p, f0 : f0 + m],
                            ones[:],
                            start=(j == 0),
                            stop=(j == len(blks) - 1),
                            skip_group_check=True,
                        )
            cnt = cpool.tile([P, B * C], f32)
            nc.scalar.copy(cnt[:], ps[:])
            nc.sync.dma_start(out[:], cnt[:])
    nc.compile()
    return nc


def _build_b(n_present):
    """n_present: tuple of per-sample present-class counts (after the host
    permuted channels present-first)."""
    import concourse.bacc as bacc
    import concourse.tile as tile
    from concourse import mybir
    from concourse.bass import broadcast_tensor_aps

    FA = mybir.ActivationFunctionType
    AL = mybir.AluOpType
    f32, bf16 = mybir.dt.float32, mybir.dt.bfloat16

    L = max(n_present)

    nc = bacc.Bacc("TRN2", num_devices=NCORES, name="loss_main_v2")
    x = nc.dram_tensor("x", [B, NCH, P, C, FCH], bf16, kind="ExternalInput")
    t = nc.dram_tensor("t", [B, NCH, P, C, FCH], bf16, kind="ExternalInput")
    cols = nc.dram_tensor("cols", [P, NCOL], f32, kind="ExternalOutput")
    # segi cols: b*24 + kind*8 + c  (kind 0 = seg, 1 = intersect, 2 = u)
    segi = nc.dram_tensor("segi", [P, 2 * 24], f32, kind="ExternalOutput")
    # f-blocks within one class chunk for the TensorE reductions
    blks = [(j * P, min(P, FCH - j * P)) for j in range((FCH + P - 1) // P)]

    with tile.TileContext(nc) as tc:
        with (
            tc.tile_pool(name="xin", bufs=2) as xin,
            tc.tile_pool(name="tin", bufs=2) as tin,
            tc.tile_pool(name="ework", bufs=2) as ework,
            tc.tile_pool(name="qwork", bufs=2) as qwork,
            tc.tile_pool(name="tqwork", bufs=2) as tqwork,
            tc.tile_pool(name="junkp", bufs=1) as junkp,
            tc.tile_pool(name="small", bufs=2) as small,
            tc.tile_pool(name="cpool", bufs=1) as cpool,
            tc.psum_pool(name="pp", bufs=2) as pp,
        ):
            acc = cpool.tile([P, 2 * 24], f32)
            nc.vector.memset(acc[:], 0.0)
            colsb = cpool.tile([P, NCOL], f32)
            nc.vector.memset(colsb[:], 0.0)
            ones = cpool.tile([P, 1], bf16)
            nc.vector.memset(ones[:], 1.0)
            # S for every chunk, so the Ln ops can run back-to-back at the
            # end (exp+ln share one act table set; keep loads to a minimum)
            S_all = cpool.tile([P, B * NCH, FCH], f32)

            for b in range(B):
                n = n_present[b]
                for ch in range(NCH):
                    x_sb = xin.tile([P, C, FCH], bf16, tag="x")
                    nc.sync.dma_start(x_sb[:], x[b, ch])
                    t_sb = tin.tile([P, n, FCH], bf16, tag="t")
                    nc.sync.dma_start(t_sb[:], t[b, ch, :, 0:n, :])

                    # ---- bg merge: absent logits folded into class 0 ----
                    if n < C:
                        na = C - n
                        if na == 1:
                            bgs = x_sb[:, n, :]
                        else:
                            bg = small.tile([P, FCH], bf16, tag="bg")
                            nc.vector.tensor_tensor(
                                out=bg[:], in0=x_sb[:, n, :],
                                in1=x_sb[:, n + 1, :], op=AL.add)
                            for a in range(n + 2, C):
                                bg2 = small.tile([P, FCH], bf16, tag="bg")
                                nc.vector.tensor_tensor(
                                    out=bg2[:], in0=bg[:],
                                    in1=x_sb[:, a, :], op=AL.add)
                                bg = bg2
                            bgs = bg[:]
                        nc.vector.tensor_tensor(
                            out=x_sb[:, 0, :], in0=x_sb[:, 0, :],
                            in1=bgs, op=AL.add)

                    # ---- e = exp(x) over present channels (one wide op) ----
                    e_sb = ework.tile([P, n, FCH], bf16, tag="e")
                    last_exp = nc.scalar.activation(
                        out=e_sb[:], in_=x_sb[:, 0:n, :], func=FA.Exp)

                    # ---- S = sum_c e_c (pairwise tree, bf16, flat) ----
                    S = S_all[:, b * NCH + ch, :]
                    if n == 8:
                        s4 = small.tile([P, 4, FCH], bf16, tag="s4")
                        nc.vector.tensor_tensor(
                            out=s4[:], in0=e_sb[:, 0:4, :],
                            in1=e_sb[:, 4:8, :], op=AL.add)
                        s2 = small.tile([P, 2, FCH], bf16, tag="s2")
                        nc.vector.tensor_tensor(
                            out=s2[:], in0=s4[:, 0:2, :],
                            in1=s4[:, 2:4, :], op=AL.add)
                        nc.vector.tensor_tensor(
                            out=S, in0=s2[:, 0, :], in1=s2[:, 1, :],
                            op=AL.add)
                    elif n == 5:
                        s2 = small.tile([P, 2, FCH], bf16, tag="s2")
                        nc.vector.tensor_tensor(
                            out=s2[:], in0=e_sb[:, 0:2, :],
                            in1=e_sb[:, 2:4, :], op=AL.add)
                        s1 = small.tile([P, FCH], bf16, tag="s1")
                        nc.vector.tensor_tensor(
                            out=s1[:], in0=s2[:, 0, :], in1=s2[:, 1, :],
                            op=AL.add)
                        nc.vector.tensor_tensor(
                            out=S, in0=s1[:], in1=e_sb[:, 4, :], op=AL.add)
                    else:
                        # generic pairwise tree
                        cur = [e_sb[:, c, :] for c in range(n)]
                        lvl = 0
                        while len(cur) > 1:
                            nxt = []
                            for i in range(0, len(cur) - 1, 2):
                                if len(cur) == 2:
                                    o = S
                                else:
                                    ot = small.tile(
                                        [P, FCH], bf16, tag=f"st{lvl}{i}")
                                    o = ot[:]
                                nc.vector.tensor_tensor(
                                    out=o, in0=cur[i], in1=cur[i + 1],
                                    op=AL.add)
                                nxt.append(o)
                            if len(cur) % 2:
                                nxt.append(cur[-1])
                            cur = nxt
                            lvl += 1

                    # ---- r = 1/S: fast-approx recip, bf16 writeback ----
                    from concourse.dve_ops import (
                        RECIP_APPROX_FAST_CONSTS, RECIPROCAL_APPROX_FAST)
                    r16 = small.tile([P, 1, FCH], bf16, tag="r16")
                    rc = RECIP_APPROX_FAST_CONSTS
                    nc.vector._custom_dve(
                        RECIPROCAL_APPROX_FAST, out=r16[:, 0, :], in0=S,
                        s0=rc["s0"], s1=rc["s1"], imm2=rc["imm2"])

                    # ---- q = e * r (wide TT, r broadcast along C) ----
                    q_sb = qwork.tile([P, n, FCH], bf16, tag="q")
                    if Q_BCAST:
                        _, rb = broadcast_tensor_aps(e_sb[:], r16[:])
                        nc.vector.tensor_tensor(
                            out=q_sb[:], in0=e_sb[:], in1=rb, op=AL.mult)
                    else:
                        for c in range(n):
                            nc.vector.tensor_tensor(
                                out=q_sb[:, c, :], in0=e_sb[:, c, :],
                                in1=r16[:, 0, :], op=AL.mult)

                    # ---- tq = t * q ----
                    tq_sb = tqwork.tile([P, n, FCH], bf16, tag="tq")
                    nc.vector.tensor_tensor(
                        out=tq_sb[:], in0=t_sb[:], in1=q_sb[:], op=AL.mult)

                    # ---- u-term product: tx = t * m (reduced on TensorE) ----
                    tx_sb = junkp.tile([P, n, FCH], bf16, tag="tx")
                    nc.vector.tensor_tensor(
                        out=tx_sb[:], in0=t_sb[:], in1=x_sb[:, 0:n, :],
                        op=AL.mult)

                    # ---- per-class seg/intersect partial sums on TensorE:
                    # lhsT = class f-block [128, m], rhs = ones -> out [m, 1]
                    # accumulated into the (kind, class) psum column.
                    ps = pp.tile([P, 24], f32, tag="ps")
                    for kind, src in ((0, q_sb), (1, tq_sb), (2, tx_sb)):
                        for c in range(n):
                            col = kind * 8 + c
                            for j, (f0, m) in enumerate(blks):
                                nc.tensor.matmul(
                                    ps[0:m, col : col + 1],
                                    src[:, c, f0 : f0 + m],
                                    ones[:],
                                    start=(j == 0),
                                    stop=(j == len(blks) - 1),
                                    skip_group_check=True,
                                )
                    # accumulate this chunk's psum columns into the SBUF acc
                    if n == C:
                        nc.vector.scalar_tensor_tensor(
                            out=acc[:, b * 24 : b * 24 + 24],
                            in0=ps[:, 0:24], scalar=1.0,
                            in1=acc[:, b * 24 : b * 24 + 24],
                            op0=AL.mult, op1=AL.add)
                    else:
                        for kind in range(3):
                            r0 = kind * 8
                            nc.vector.scalar_tensor_tensor(
                                out=acc[:, b * 24 + r0 : b * 24 + r0 + n],
                                in0=ps[:, r0 : r0 + n], scalar=1.0,
                                in1=acc[:, b * 24 + r0 : b * 24 + r0 + n],
                                op0=AL.mult, op1=AL.add)

            # ---- CE lse terms at the end: sum(ln(S + pad_b)) via ACT ----
            junk_ln = cpool.tile([P, FCH], f32)
            padc = cpool.tile([P, B], f32)
            for b in range(B):
                nc.vector.memset(padc[:, b : b + 1], float(L - n_present[b]))
            from concourse.tile import add_dep_helper
            for b in range(B):
                for ch in range(NCH):
                    lcol = b * NCH + ch
                    ln_inst = nc.scalar.activation(
                        out=junk_ln[:], in_=S_all[:, b * NCH + ch, :],
                        func=FA.Ln, bias=padc[:, b : b + 1], scale=1.0,
                        accum_out=colsb[:, lcol : lcol + 1])
                    # keep every Ln after the final Exp: one table switch
                    add_dep_helper(ln_inst.ins, last_exp.ins, False,
                                   "batch ln after exps")

            nc.sync.dma_start(cols[:], colsb[:])
            nc.sync.dma_start(segi[:], acc[:])
    nc.compile()
    return nc


def _get(key, builder, *args):
    if key not in _CACHE:
        _CACHE[key] = builder(*args)
    return _CACHE[key]


def _run(nc, in_maps, out_names):
    if os.environ.get("K_SIM", "0") == "1":
        import concourse.bass_interp as bass_interp
        sim = bass_interp.MultiCoreSim(nc, NCORES)
        for k in range(NCORES):
            for name, arr in in_maps[k].items():
                sim.cores[k].tensor(name)[:] = arr
        sim.simulate()
        return [{o: sim.cores[k].tensor(o).copy() for o in out_names}
                for k in range(NCORES)]
    from concourse.bass_utils import run_bass_kernel_spmd
    return run_bass_kernel_spmd(
        nc, in_maps, core_ids=list(range(NCORES))).results


def run_a(t8maps):
    nc = _get("a", _build_a)
    results = _run(nc, [{"t8": tk} for tk in t8maps], ["cnt"])
    cnt_g = np.zeros((B, C), dtype=np.float64)
    for r in results:
        cnt_g += r["cnt"].astype(np.float64).sum(axis=0).reshape(B, C)
    return cnt_g


def run_b(xmaps, tmaps, n_present):
    nc = _get(("b", tuple(n_present)), _build_b, tuple(n_present))
    in_maps = [{"x": xmaps[k], "t": tmaps[k]} for k in range(NCORES)]
    results = _run(nc, in_maps, ["cols", "segi"])
    cols = np.zeros((NCOL,), dtype=np.float64)
    segi = np.zeros((2 * 24,), dtype=np.float64)
    for r in results:
        cols += r["cols"].astype(np.float64).sum(axis=0)
        segi += r["segi"].astype(np.float64).sum(axis=0)
    return cols, segi


def kernel(net_output, target):
    xs = np.asarray(net_output)
    ts = np.asarray(target)

    # -------- launch A: counts (t as fp8; 0/1 exact) --------
    # per-core layout [P, B*C, FREE]: 25.6KB contiguous per partition row
    t5 = ts.reshape(B, C, NCORES, P, FREE).transpose(2, 3, 0, 1, 4)
    t8maps = [np.ascontiguousarray(t5[k].reshape(P, B * C, FREE)).astype(
        ml_dtypes.float8_e4m3fn) for k in range(NCORES)]
    cnt_g = run_a(t8maps)

    present = cnt_g > 0.5
    perms = []
    n_present = []
    for b in range(B):
        pr = np.nonzero(present[b])[0]
        ab = np.nonzero(~present[b])[0]
        perms.append(np.concatenate([pr, ab]).astype(np.int64))
        n_present.append(int(len(pr)))
    n_present = tuple(n_present)
    L = max(n_present)

    # -------- launch B inputs: permuted present-first, chunked bf16 --------
    x6 = xs.reshape(B, C, NCORES, P, NCH, FCH)
    t6 = ts.reshape(B, C, NCORES, P, NCH, FCH)
    xp = np.stack([x6[b, perms[b]] for b in range(B)])  # [B,C,K,P,NCH,FCH]
    tp = np.stack([t6[b, perms[b]] for b in range(B)])
    # -> [K, B, NCH, P, C, FCH]
    xp = np.ascontiguousarray(
        xp.transpose(2, 0, 4, 3, 1, 5)).astype(ml_dtypes.bfloat16)
    tp = np.ascontiguousarray(
        tp.transpose(2, 0, 4, 3, 1, 5)).astype(ml_dtypes.bfloat16)
    xmaps = [xp[k] for k in range(NCORES)]
    tmaps = [tp[k] for k in range(NCORES)]

    cols, segi = run_b(xmaps, tmaps, n_present)

    # -------- host finish --------
    segs = segi.reshape(B, 3, 8)
    lse_sum = cols.sum()
    u_sum = segs[:, 2, :].sum()
    ce = (lse_sum - u_sum) / NVOX

    dice_is = []
    for b in range(B):
        n = n_present[b]
        seg = segs[b, 0, :n]
        inter = segs[b, 1, :n]
        cnt = cnt_g[b, perms[b][:n]]
        dice_c = 2.0 * inter / (cnt + seg + 1e-5)
        dice_is.append(1.0 - dice_c.sum() / n)
    dc = np.mean(dice_is)
    return np.asarray(0.5 * ce + 0.5 * dc, dtype=np.float32)
